# revision 8
# baseline (speedup 1.0000x reference)
"""CfC (closed-form continuous-time) RNN cell scanned over T=1024 steps.

Data-parallel across 8 NeuronCores: batch 64 -> 8 per core, weights replicated.
Per-core, per-step structure (batch-major [8, units] tiles):
  stage A : bb_raw = [x_t, h] @ Wbb       (6 bf16 matmuls, N=512, into PSUM)
  silu    : transpose bb_raw -> unit-major [128, 4x8], sigmoid+mul there
            (cheap ACT/DVE: free dim 32 instead of 512), cast to bf16 = lhsT
  stage B : ff1r/ff2r/tar/tbr = bb @ W*   (16 bf16 matmuls into one 4-bank PSUM)
  combine : tanh(ff1r||ff2r), t_sum = tar*w_ts + tbr, s += t_sum,
            h = ff1 + sigmoid(s)*(ff2-ff1)   (DVE/ACT, batch-major)
  hT      : 4 PE-transposes of h -> [128, 4, 8] bf16 lhsT for next step
x @ Wbb_x is NOT precomputed; instead x is cast to bf16 in DRAM once and
xbar-transposed per 64-step block so x_t^T feeds stage A directly.
w_ts = exp(r*(1-2*ln r)), r = 1/ts, precomputed on-chip before the loop
(Log/Exp table set), loop itself only uses Sigmoid/Tanh (one table set).
All matmul inputs bf16; PSUM accumulation, state s and elementwise fp32.
"""

import numpy as np

import concourse.bass as bass
import concourse.bacc as bacc_mod
import concourse.mybir as mybir
import concourse.tile as tile
from concourse.bass_utils import run_bass_kernel_spmd
from concourse.masks import make_identity

AF = mybir.ActivationFunctionType
ALU = mybir.AluOpType
F32 = mybir.dt.float32
BF16 = mybir.dt.bfloat16

B, T_FULL, D, H, BU = 64, 1024, 256, 512, 512
NCORES = 8
BL = B // NCORES          # 8 batch rows per core
TBLK = 64                 # time steps per DMA block
KD = D // 128             # 2  x-part k-chunks
KH = H // 128             # 4  h-part k-chunks
KB = BU // 128            # 4  stage-B k-chunks
NT = 4                    # output tensors: ff1, ff2, ta, tb

T_STEPS = T_FULL          # tests may shrink this (must stay %TBLK==0)
TRACE = False             # tests may enable profiling


def _build(nc: bass.Bass, Ts: int, bias_any: dict):
    NBLK = Ts // TBLK

    x_e = nc.declare_dram_parameter("x", [BL, T_FULL, D], F32, isOutput=False)
    ts_e = nc.declare_dram_parameter("ts", [BL, T_FULL], F32, isOutput=False)
    h0_e = nc.declare_dram_parameter("h0", [BL, H], F32, isOutput=False)
    s0_e = nc.declare_dram_parameter("s0", [BL, H], F32, isOutput=False)
    wbb_e = nc.declare_dram_parameter("Wbb", [D + H, BU], F32, isOutput=False)
    bbb_e = nc.declare_dram_parameter("bbb", [BU], F32, isOutput=False)
    wo_e = [
        nc.declare_dram_parameter(n, [BU, H], F32, isOutput=False)
        for n in ("Wff1", "Wff2", "Wta", "Wtb")
    ]
    bo_e = [
        nc.declare_dram_parameter(n, [H], F32, isOutput=False)
        for n in ("bff1", "bff2", "bta", "btb")
    ]
    out_e = nc.declare_dram_parameter("out", [BL, T_FULL, H], F32, isOutput=True)

    xbf = nc.dram_tensor("xbf", (BL, T_FULL, D), BF16)  # bf16 copy of x

    with tile.TileContext(nc) as tc:
        import contextlib

        ctx = contextlib.ExitStack()
        with ctx:
            const = ctx.enter_context(tc.tile_pool(name="const", bufs=1))
            xtp = ctx.enter_context(tc.tile_pool(name="xtp", bufs=2))
            outp = ctx.enter_context(tc.tile_pool(name="outp", bufs=2))
            lhsp = ctx.enter_context(tc.tile_pool(name="lhsp", bufs=2))
            sp = ctx.enter_context(tc.tile_pool(name="sp", bufs=2))
            spool = ctx.enter_context(tc.tile_pool(name="spool", bufs=2))
            psA_p = ctx.enter_context(tc.tile_pool(name="psA", bufs=2, space="PSUM"))
            psB_p = ctx.enter_context(tc.tile_pool(name="psB", bufs=1, space="PSUM"))
            psT_p = ctx.enter_context(tc.tile_pool(name="psT", bufs=2, space="PSUM"))

            # ---- constants / weights (bf16, cast during SWDGE DMA) ----
            ident8 = const.tile([8, 8], F32, name="ident8")
            make_identity(nc, ident8)

            wbb_sb = const.tile([128, KD + KH, BU], BF16, name="wbb_sb")
            nc.gpsimd.dma_start(
                out=wbb_sb, in_=wbb_e[:].rearrange("(ko p) n -> p ko n", p=128)
            )
            wout_sb = const.tile([128, KB, NT, H], BF16, name="wout_sb")
            for t_i in range(NT):
                nc.gpsimd.dma_start(
                    out=wout_sb[:, :, t_i, :],
                    in_=wo_e[t_i][:].rearrange("(ko p) n -> p ko n", p=128),
                )

            # optional biases (inputs are zeros in this problem -> usually skipped)
            bbbT_sb = None
            if bias_any["bbb"]:
                bbbT_sb = const.tile([128, KB], F32, name="bbbT_sb")
                nc.sync.dma_start(
                    out=bbbT_sb, in_=bbb_e[:].rearrange("(ko p) -> p ko", p=128)
                )
            bout_sb = None
            if any(bias_any[n] for n in ("bff1", "bff2", "bta", "btb")):
                bout_sb = const.tile([1, NT, H], F32, name="bout_sb")
                for t_i in range(NT):
                    nc.sync.dma_start(out=bout_sb[:, t_i, :], in_=bo_e[t_i][None, :])

            # ---- x -> bf16 DRAM copy (one big casting DMA) ----
            nc.gpsimd.dma_start(out=xbf[:, :Ts, :], in_=x_e[:, :Ts, :])

            # ---- w_ts = exp(r - 2 r ln r), r = 1/ts  (r > 0 always) ----
            wts_sb = const.tile([BL, T_FULL], F32, name="wts_sb")
            tsr = const.tile([BL, T_FULL], F32, name="tsr")
            nc.sync.dma_start(out=tsr[:, :Ts], in_=ts_e[:, :Ts])
            nc.vector.reciprocal(out=tsr[:, :Ts], in_=tsr[:, :Ts])
            lg = const.tile([BL, T_FULL], F32, name="lg")
            nc.scalar.activation(out=lg[:, :Ts], in_=tsr[:, :Ts], func=AF.Ln)
            nc.vector.tensor_mul(out=lg[:, :Ts], in0=lg[:, :Ts], in1=tsr[:, :Ts])
            nc.vector.tensor_scalar_mul(out=lg[:, :Ts], in0=lg[:, :Ts], scalar1=-2.0)
            nc.vector.tensor_add(out=lg[:, :Ts], in0=lg[:, :Ts], in1=tsr[:, :Ts])
            nc.scalar.activation(out=wts_sb[:, :Ts], in_=lg[:, :Ts], func=AF.Exp)

            # ---- initial state ----
            s_cur = spool.tile([BL, H], F32, tag="s", name="s_init")
            nc.sync.dma_start(out=s_cur, in_=s0_e[:])
            h0b = const.tile([BL, H], F32, name="h0b")
            nc.sync.dma_start(out=h0b, in_=h0_e[:])
            psTh0 = psT_p.tile([128, KH, BL], F32, tag="psT", name="psTh0")
            for j in range(KH):
                nc.tensor.transpose(
                    psTh0[:, j, :], h0b[:, j * 128 : (j + 1) * 128], ident8
                )
            hT_cur = lhsp.tile([128, KH, BL], BF16, tag="hT", name="hT_init")
            nc.vector.tensor_copy(out=hT_cur, in_=psTh0)

            xbf3 = xbf[:]  # [BL, T, D]

            for blk in range(NBLK):
                t0 = blk * TBLK
                # x^T for this block via xbar transpose: [128, ko, b*TBLK+t]
                xT = xtp.tile([128, KD, BL * TBLK], BF16, tag="xT", name="xT")
                for b in range(BL):
                    for ko in range(KD):
                        nc.sync.dma_start_transpose(
                            xT[:, ko, b * TBLK : (b + 1) * TBLK],
                            xbf3[b, t0 : t0 + TBLK, ko * 128 : (ko + 1) * 128],
                        )
                xTv = xT.rearrange("p ko (b t) -> p ko t b", t=TBLK)

                for ti in range(TBLK):
                    t = t0 + ti
                    # ---- stage A: bb_raw = [x_t, h] @ Wbb ----
                    psA = psA_p.tile([BL, BU], F32, tag="psA", name="psA")
                    for ko in range(KD):
                        nc.tensor.matmul(
                            psA,
                            lhsT=xTv[:, ko, ti, :],
                            rhs=wbb_sb[:, ko, :],
                            start=(ko == 0),
                            stop=False,
                        )
                    for j in range(KH):
                        nc.tensor.matmul(
                            psA,
                            lhsT=hT_cur[:, j, :],
                            rhs=wbb_sb[:, KD + j, :],
                            start=False,
                            stop=(j == KH - 1),
                        )

                    # ---- silu in transposed space -> bbT (bf16 lhsT) ----
                    abm = sp.tile([BL, BU], F32, tag="abm", name="abm")
                    nc.vector.tensor_copy(out=abm, in_=psA)
                    psTb = psT_p.tile([128, KB, BL], F32, tag="psT", name="psTb")
                    for j in range(KB):
                        nc.tensor.transpose(
                            psTb[:, j, :], abm[:, j * 128 : (j + 1) * 128], ident8
                        )
                    if bbbT_sb is not None:
                        nc.vector.tensor_tensor(
                            psTb,
                            psTb,
                            bbbT_sb[:, :, None].to_broadcast(psTb.shape),
                            ALU.add,
                        )
                    sgT = sp.tile([128, KB, BL], F32, tag="sgT", name="sgT")
                    nc.scalar.activation(out=sgT, in_=psTb, func=AF.Sigmoid)
                    bbT = lhsp.tile([128, KB, BL], BF16, tag="bbT", name="bbT")
                    nc.vector.tensor_mul(out=bbT, in0=psTb, in1=sgT)

                    # ---- stage B: 4 output matmuls ----
                    psB = psB_p.tile([BL, NT, H], F32, tag="psB", name="psB")
                    for t_i in range(NT):
                        for j in range(KB):
                            nc.tensor.matmul(
                                psB[:, t_i, :],
                                lhsT=bbT[:, j, :],
                                rhs=wout_sb[:, j, t_i, :],
                                start=(j == 0),
                                stop=(j == KB - 1),
                            )
                    if bout_sb is not None:
                        nc.vector.tensor_tensor(
                            psB, psB, bout_sb.to_broadcast(psB.shape), ALU.add
                        )

                    # ---- tanh(ff1r || ff2r) ----
                    ff = sp.tile([BL, 2, H], F32, tag="ff", name="ff")
                    nc.scalar.activation(out=ff, in_=psB[:, 0:2, :], func=AF.Tanh)

                    # ---- s += ta*w_ts + tb ----
                    tsum = sp.tile([BL, H], F32, tag="tsum", name="tsum")
                    nc.vector.tensor_scalar_mul(
                        out=tsum, in0=psB[:, 2, :], scalar1=wts_sb[:, t : t + 1]
                    )
                    s_new = spool.tile([BL, H], F32, tag="s", name="s_new")
                    nc.vector.tensor_add(out=s_new, in0=tsum, in1=psB[:, 3, :])
                    nc.vector.tensor_add(out=s_new, in0=s_new, in1=s_cur)
                    s_cur = s_new

                    # ---- h = ff1 + sigmoid(s)*(ff2-ff1) ----
                    tint = sp.tile([BL, H], F32, tag="tint", name="tint")
                    nc.scalar.activation(out=tint, in_=s_new, func=AF.Sigmoid)
                    dif = sp.tile([BL, H], F32, tag="dif", name="dif")
                    nc.vector.tensor_tensor(dif, ff[:, 1, :], ff[:, 0, :], ALU.subtract)
                    hb = outp.tile([BL, H], F32, tag="hb", name="hb", bufs=3)
                    nc.vector.tensor_mul(out=hb, in0=tint, in1=dif)
                    nc.vector.tensor_add(out=hb, in0=hb, in1=ff[:, 0, :])
                    nc.sync.dma_start(out=out_e[:, t, :], in_=hb)

                    # ---- h^T (bf16) for next step ----
                    psTh = psT_p.tile([128, KH, BL], F32, tag="psT", name="psTh")
                    for j in range(KH):
                        nc.tensor.transpose(
                            psTh[:, j, :], hb[:, j * 128 : (j + 1) * 128], ident8
                        )
                    hT_cur = lhsp.tile([128, KH, BL], BF16, tag="hT", name="hT")
                    nc.vector.tensor_copy(out=hT_cur, in_=psTh)
    return nc


def kernel(**inputs):
    arr = {k: np.ascontiguousarray(np.asarray(v, dtype=np.float32)) for k, v in inputs.items()}
    bias_any = {
        n: bool(np.any(arr[n] != 0.0)) for n in ("bbb", "bff1", "bff2", "bta", "btb")
    }

    nc = bacc_mod.Bacc()
    _build(nc, T_STEPS, bias_any)
    nc.compile()

    shared = {
        k: arr[k]
        for k in ("Wbb", "bbb", "Wff1", "bff1", "Wff2", "bff2", "Wta", "bta", "Wtb", "btb")
    }
    in_maps = []
    for c in range(NCORES):
        sl = slice(c * BL, (c + 1) * BL)
        m = dict(shared)
        m["x"] = np.ascontiguousarray(arr["x"][sl])
        m["ts"] = np.ascontiguousarray(arr["ts"][sl])
        m["h0"] = np.ascontiguousarray(arr["h0"][sl])
        m["s0"] = np.ascontiguousarray(arr["s0"][sl])
        in_maps.append(m)

    res = run_bass_kernel_spmd(nc, in_maps, core_ids=list(range(NCORES)), trace=TRACE)
    if TRACE:
        kernel.last_results = res
    out = np.concatenate([r["out"] for r in res.results], axis=0)
    return out[:, :T_STEPS, :] if T_STEPS != T_FULL else out


# revision 9
# speedup vs baseline: 57.5765x; 57.5765x over previous
"""CfC (closed-form continuous-time) RNN cell scanned over T=1024 steps.

Data-parallel across 8 NeuronCores: batch 64 -> 8 per core, weights replicated.
Per-core, per-step structure (batch-major [8, units] tiles):
  stage A : bb_raw = [x_t, h] @ Wbb       (6 bf16 matmuls, N=512, into PSUM)
  silu    : transpose bb_raw -> unit-major [128, 4x8], sigmoid+mul there
            (cheap ACT/DVE: free dim 32 instead of 512), cast to bf16 = lhsT
  stage B : ff1r/ff2r/tar/tbr = bb @ W*   (16 bf16 matmuls into one 4-bank PSUM)
  combine : tanh(ff1r||ff2r), t_sum = tar*w_ts + tbr, s += t_sum,
            h = ff1 + sigmoid(s)*(ff2-ff1)   (DVE/ACT, batch-major)
  hT      : 4 PE-transposes of h -> [128, 4, 8] bf16 lhsT for next step
x @ Wbb_x is NOT precomputed; instead x is cast to bf16 in DRAM once and
xbar-transposed per 64-step block so x_t^T feeds stage A directly.
w_ts = exp(r*(1-2*ln r)), r = 1/ts, precomputed on-chip before the loop
(Log/Exp table set), loop itself only uses Sigmoid/Tanh (one table set).
All matmul inputs bf16; PSUM accumulation, state s and elementwise fp32.
"""

import numpy as np

import concourse.bass as bass
import concourse.bacc as bacc_mod
import concourse.mybir as mybir
import concourse.tile as tile
from concourse.bass_utils import run_bass_kernel_spmd
from concourse.masks import make_identity

AF = mybir.ActivationFunctionType
ALU = mybir.AluOpType
F32 = mybir.dt.float32
BF16 = mybir.dt.bfloat16

B, T_FULL, D, H, BU = 64, 1024, 256, 512, 512
NCORES = 8
BL = B // NCORES          # 8 batch rows per core
TBLK = 64                 # time steps per DMA block
KD = D // 128             # 2  x-part k-chunks
KH = H // 128             # 4  h-part k-chunks
KB = BU // 128            # 4  stage-B k-chunks
NT = 4                    # output tensors: ff1, ff2, ta, tb

T_STEPS = T_FULL          # tests may shrink this (must stay %TBLK==0)
TRACE = False             # tests may enable profiling


def _build(nc: bass.Bass, Ts: int, bias_any: dict):
    NBLK = Ts // TBLK

    x_e = nc.declare_dram_parameter("x", [BL, T_FULL, D], F32, isOutput=False)
    ts_e = nc.declare_dram_parameter("ts", [BL, T_FULL], F32, isOutput=False)
    h0_e = nc.declare_dram_parameter("h0", [BL, H], F32, isOutput=False)
    s0_e = nc.declare_dram_parameter("s0", [BL, H], F32, isOutput=False)
    wbb_e = nc.declare_dram_parameter("Wbb", [D + H, BU], F32, isOutput=False)
    bbb_e = nc.declare_dram_parameter("bbb", [BU], F32, isOutput=False)
    wo_e = [
        nc.declare_dram_parameter(n, [BU, H], F32, isOutput=False)
        for n in ("Wff1", "Wff2", "Wta", "Wtb")
    ]
    bo_e = [
        nc.declare_dram_parameter(n, [H], F32, isOutput=False)
        for n in ("bff1", "bff2", "bta", "btb")
    ]
    out_e = nc.declare_dram_parameter("out", [BL, T_FULL, H], F32, isOutput=True)

    xbf = nc.dram_tensor("xbf", (BL, T_FULL, D), BF16)  # bf16 copy of x

    with tile.TileContext(nc) as tc:
        import contextlib

        ctx = contextlib.ExitStack()
        with ctx:
            const = ctx.enter_context(tc.tile_pool(name="const", bufs=1))
            xtp = ctx.enter_context(tc.tile_pool(name="xtp", bufs=2))
            outp = ctx.enter_context(tc.tile_pool(name="outp", bufs=2))
            lhsp = ctx.enter_context(tc.tile_pool(name="lhsp", bufs=2))
            sp = ctx.enter_context(tc.tile_pool(name="sp", bufs=2))
            spool = ctx.enter_context(tc.tile_pool(name="spool", bufs=2))
            psA_p = ctx.enter_context(tc.tile_pool(name="psA", bufs=2, space="PSUM"))
            psB_p = ctx.enter_context(tc.tile_pool(name="psB", bufs=1, space="PSUM"))
            psT_p = ctx.enter_context(tc.tile_pool(name="psT", bufs=2, space="PSUM"))

            # ---- constants / weights (bf16, cast during SWDGE DMA) ----
            ident8 = const.tile([8, 8], F32, name="ident8")
            make_identity(nc, ident8)

            wbb_sb = const.tile([128, KD + KH, BU], BF16, name="wbb_sb")
            nc.gpsimd.dma_start(
                out=wbb_sb, in_=wbb_e[:].rearrange("(ko p) n -> p ko n", p=128)
            )
            wout_sb = const.tile([128, KB, NT, H], BF16, name="wout_sb")
            for t_i in range(NT):
                nc.gpsimd.dma_start(
                    out=wout_sb[:, :, t_i, :],
                    in_=wo_e[t_i][:].rearrange("(ko p) n -> p ko n", p=128),
                )

            # optional biases (inputs are zeros in this problem -> usually skipped)
            bbbT_sb = None
            if bias_any["bbb"]:
                bbbT_sb = const.tile([128, KB], F32, name="bbbT_sb")
                nc.sync.dma_start(
                    out=bbbT_sb, in_=bbb_e[:].rearrange("(ko p) -> p ko", p=128)
                )
            bout_sb = None
            if any(bias_any[n] for n in ("bff1", "bff2", "bta", "btb")):
                bout_sb = const.tile([1, NT, H], F32, name="bout_sb")
                for t_i in range(NT):
                    nc.sync.dma_start(out=bout_sb[:, t_i, :], in_=bo_e[t_i][None, :])

            # ---- x -> bf16 DRAM copy (one big casting DMA) ----
            nc.gpsimd.dma_start(out=xbf[:, :Ts, :], in_=x_e[:, :Ts, :])

            # ---- w_ts = exp(r - 2 r ln r), r = 1/ts  (r > 0 always) ----
            wts_sb = const.tile([BL, T_FULL], F32, name="wts_sb")
            tsr = const.tile([BL, T_FULL], F32, name="tsr")
            nc.sync.dma_start(out=tsr[:, :Ts], in_=ts_e[:, :Ts])
            nc.vector.reciprocal(out=tsr[:, :Ts], in_=tsr[:, :Ts])
            lg = const.tile([BL, T_FULL], F32, name="lg")
            nc.scalar.activation(out=lg[:, :Ts], in_=tsr[:, :Ts], func=AF.Ln)
            nc.vector.tensor_mul(out=lg[:, :Ts], in0=lg[:, :Ts], in1=tsr[:, :Ts])
            nc.vector.tensor_scalar_mul(out=lg[:, :Ts], in0=lg[:, :Ts], scalar1=-2.0)
            nc.vector.tensor_add(out=lg[:, :Ts], in0=lg[:, :Ts], in1=tsr[:, :Ts])
            nc.scalar.activation(out=wts_sb[:, :Ts], in_=lg[:, :Ts], func=AF.Exp)

            # ---- initial state ----
            s_cur = spool.tile([BL, H], F32, tag="s", name="s_init")
            nc.sync.dma_start(out=s_cur, in_=s0_e[:])
            h0b = const.tile([BL, H], F32, name="h0b")
            nc.sync.dma_start(out=h0b, in_=h0_e[:])
            psTh0 = psT_p.tile([128, KH, BL], F32, tag="psT", name="psTh0")
            for j in range(KH):
                nc.tensor.transpose(
                    psTh0[:, j, :], h0b[:, j * 128 : (j + 1) * 128], ident8
                )
            hT_cur = lhsp.tile([128, KH, BL], BF16, tag="hT", name="hT_init")
            nc.vector.tensor_copy(out=hT_cur, in_=psTh0)

            xbf3 = xbf[:]  # [BL, T, D]

            for blk in range(NBLK):
                t0 = blk * TBLK
                # x^T for this block via xbar transpose: [128, ko, b*TBLK+t]
                xT = xtp.tile([128, KD, BL * TBLK], BF16, tag="xT", name="xT")
                for b in range(BL):
                    for ko in range(KD):
                        nc.sync.dma_start_transpose(
                            xT[:, ko, b * TBLK : (b + 1) * TBLK],
                            xbf3[b, t0 : t0 + TBLK, ko * 128 : (ko + 1) * 128],
                        )
                xTv = xT.rearrange("p ko (b t) -> p ko t b", t=TBLK)

                for ti in range(TBLK):
                    t = t0 + ti
                    # ---- stage A: bb_raw = [x_t, h] @ Wbb ----
                    psA = psA_p.tile([BL, BU], F32, tag="psA", name="psA")
                    for ko in range(KD):
                        nc.tensor.matmul(
                            psA,
                            lhsT=xTv[:, ko, ti, :],
                            rhs=wbb_sb[:, ko, :],
                            start=(ko == 0),
                            stop=False,
                        )
                    for j in range(KH):
                        nc.tensor.matmul(
                            psA,
                            lhsT=hT_cur[:, j, :],
                            rhs=wbb_sb[:, KD + j, :],
                            start=False,
                            stop=(j == KH - 1),
                        )

                    # ---- silu in transposed space -> bbT (bf16 lhsT) ----
                    abm = sp.tile([BL, BU], F32, tag="abm", name="abm")
                    nc.vector.tensor_copy(out=abm, in_=psA)
                    psTb = psT_p.tile([128, KB, BL], F32, tag="psT", name="psTb")
                    for j in range(KB):
                        nc.tensor.transpose(
                            psTb[:, j, :], abm[:, j * 128 : (j + 1) * 128], ident8
                        )
                    if bbbT_sb is not None:
                        nc.vector.tensor_tensor(
                            psTb,
                            psTb,
                            bbbT_sb[:, :, None].to_broadcast(psTb.shape),
                            ALU.add,
                        )
                    sgT = sp.tile([128, KB, BL], F32, tag="sgT", name="sgT")
                    nc.scalar.activation(out=sgT, in_=psTb, func=AF.Sigmoid)
                    bbT = lhsp.tile([128, KB, BL], BF16, tag="bbT", name="bbT")
                    nc.vector.tensor_mul(out=bbT, in0=psTb, in1=sgT)

                    # ---- stage B: 4 output matmuls ----
                    psB = psB_p.tile([BL, NT, H], F32, tag="psB", name="psB")
                    for t_i in range(NT):
                        for j in range(KB):
                            nc.tensor.matmul(
                                psB[:, t_i, :],
                                lhsT=bbT[:, j, :],
                                rhs=wout_sb[:, j, t_i, :],
                                start=(j == 0),
                                stop=(j == KB - 1),
                            )
                    if bout_sb is not None:
                        nc.vector.tensor_tensor(
                            psB, psB, bout_sb.to_broadcast(psB.shape), ALU.add
                        )

                    # ---- tanh(ff1r || ff2r) ----
                    ff = sp.tile([BL, 2, H], F32, tag="ff", name="ff")
                    nc.scalar.activation(out=ff, in_=psB[:, 0:2, :], func=AF.Tanh)

                    # ---- s += ta*w_ts + tb ----
                    tsum = sp.tile([BL, H], F32, tag="tsum", name="tsum")
                    nc.vector.tensor_scalar_mul(
                        out=tsum, in0=psB[:, 2, :], scalar1=wts_sb[:, t : t + 1]
                    )
                    s_new = spool.tile([BL, H], F32, tag="s", name="s_new")
                    nc.vector.tensor_add(out=s_new, in0=tsum, in1=psB[:, 3, :])
                    nc.vector.tensor_add(out=s_new, in0=s_new, in1=s_cur)
                    s_cur = s_new

                    # ---- h = ff1 + sigmoid(s)*(ff2-ff1) ----
                    tint = sp.tile([BL, H], F32, tag="tint", name="tint")
                    nc.scalar.activation(out=tint, in_=s_new, func=AF.Sigmoid)
                    dif = sp.tile([BL, H], F32, tag="dif", name="dif")
                    nc.vector.tensor_tensor(dif, ff[:, 1, :], ff[:, 0, :], ALU.subtract)
                    hb = outp.tile([BL, H], F32, tag="hb", name="hb", bufs=3)
                    nc.vector.tensor_mul(out=hb, in0=tint, in1=dif)
                    nc.vector.tensor_add(out=hb, in0=hb, in1=ff[:, 0, :])
                    nc.sync.dma_start(out=out_e[:, t, :], in_=hb)

                    # ---- h^T (bf16) for next step ----
                    psTh = psT_p.tile([128, KH, BL], F32, tag="psT", name="psTh")
                    for j in range(KH):
                        nc.tensor.transpose(
                            psTh[:, j, :], hb[:, j * 128 : (j + 1) * 128], ident8
                        )
                    hT_cur = lhsp.tile([128, KH, BL], BF16, tag="hT", name="hT")
                    nc.vector.tensor_copy(out=hT_cur, in_=psTh)
    return nc


def kernel(**inputs):
    arr = {k: np.ascontiguousarray(np.asarray(v, dtype=np.float32)) for k, v in inputs.items()}
    bias_any = {
        n: bool(np.any(arr[n] != 0.0)) for n in ("bbb", "bff1", "bff2", "bta", "btb")
    }

    nc = bacc_mod.Bacc()
    _build(nc, T_STEPS, bias_any)
    nc.compile()

    shared = {
        k: arr[k]
        for k in ("Wbb", "bbb", "Wff1", "bff1", "Wff2", "bff2", "Wta", "bta", "Wtb", "btb")
    }
    in_maps = []
    for c in range(NCORES):
        sl = slice(c * BL, (c + 1) * BL)
        m = dict(shared)
        m["x"] = np.ascontiguousarray(arr["x"][sl])
        m["ts"] = np.ascontiguousarray(arr["ts"][sl])
        m["h0"] = np.ascontiguousarray(arr["h0"][sl])
        m["s0"] = np.ascontiguousarray(arr["s0"][sl])
        in_maps.append(m)

    res = run_bass_kernel_spmd(nc, in_maps, core_ids=list(range(NCORES)))
    kernel.last_nc = nc
    kernel.last_in_maps = in_maps
    out = np.concatenate([r["out"] for r in res.results], axis=0)
    return out[:, :T_STEPS, :] if T_STEPS != T_FULL else out


def timed_run(nc, in_maps, n_warm=2, n_iter=5):
    """Steady-state wall-clock of the sharded PJRT executable (inputs resident
    on device, no output donation, no D2H in the timed region)."""
    import time

    import jax
    import jax.numpy as jnp
    from jax.sharding import Mesh, PartitionSpec
    from jax.experimental.shard_map import shard_map

    import concourse.bass2jax as b2j

    b2j.install_neuronx_cc_hook()
    partition_name = nc.partition_id_tensor.name if nc.partition_id_tensor else None
    in_names, out_names, out_avals = [], [], []
    for alloc in nc.m.functions[0].allocations:
        if not isinstance(alloc, mybir.MemoryLocationSet):
            continue
        name = alloc.memorylocations[0].name
        if alloc.kind == "ExternalInput":
            if name != partition_name:
                in_names.append(name)
        elif alloc.kind == "ExternalOutput":
            out_names.append(name)
            out_avals.append(
                jax.core.ShapedArray(tuple(alloc.tensor_shape), mybir.dt.np(alloc.dtype))
            )
    n_params = len(in_names)
    all_names = in_names + out_names
    if partition_name is not None:
        all_names.append(partition_name)

    def _body(*args):
        operands = list(args)
        if partition_name is not None:
            operands.append(b2j.partition_id_tensor())
        return tuple(
            b2j._bass_exec_p.bind(
                *operands,
                out_avals=tuple(out_avals),
                in_names=tuple(all_names),
                out_names=tuple(out_names),
                lowering_input_output_aliases=(),
                sim_require_finite=True,
                sim_require_nnan=True,
                nc=nc,
            )
        )

    devices = jax.devices()[:NCORES]
    mesh = Mesh(np.asarray(devices), ("core",))
    nin = n_params + len(out_names)
    fn = jax.jit(
        shard_map(
            _body,
            mesh=mesh,
            in_specs=(PartitionSpec("core"),) * nin,
            out_specs=(PartitionSpec("core"),) * len(out_names),
            check_rep=False,
        ),
        keep_unused=True,
    )
    sh = jax.sharding.NamedSharding(mesh, PartitionSpec("core"))
    args = [
        jax.device_put(
            np.concatenate([np.asarray(m[nm]) for m in in_maps], axis=0), sh
        )
        for nm in in_names
    ]
    for av in out_avals:
        args.append(
            jax.device_put(
                np.zeros((NCORES * av.shape[0],) + av.shape[1:], av.dtype), sh
            )
        )
    for _ in range(n_warm):
        jax.block_until_ready(fn(*args))
    times = []
    for _ in range(n_iter):
        t0 = time.perf_counter()
        jax.block_until_ready(fn(*args))
        times.append(time.perf_counter() - t0)
    return times
